# revision 46
# baseline (speedup 1.0000x reference)
"""Paged GQA decode attention (B=64, HQ=32, HKV=8, D=128) on 8 TRN2 NeuronCores.

Strategy: data-parallel over request PIECES with host-side bin packing and
per-request mixed precision.
 - Each core runs the same static program: a sequence of slots with sizes
   `pattern` (in 128-token chunks). A slot accumulates attention for ONE
   contiguous piece of one request; long requests split into several pieces
   (across slots and cores) and the host sums the partial acc/den afterwards,
   which softmax's linearity over a shared exp-shift makes exact.
 - Bin packing (largest-bins-first, split-largest / exact-fit greedy over a
   locally-searched slot-size pattern) brings the per-core chunk count to the
   ceil(total/8) optimum, minimizing DMA bytes.
 - The longest F8_REQS requests carry K and V in fp8e4m3 (their many-token
   softmax averages quantization noise away; q and E stay bf16 via
   mixed-dtype matmuls), packed into dedicated fp8 slots interleaved with
   the bf16 slots so every DMA group has a similar byte mix.
 - Host gathers each piece's KV blocks (honoring block_tables) into per-core
   shards: K pre-transposed to [d, l] tiles, V natural [l, d]. The token
   validity mask is folded into V host-side (invalid rows zeroed) and each
   kv head's V carries a 129th column holding the validity indicator, so the
   PV matmul also accumulates the softmax denominator and exp needs no
   bias/mask at all.
 - Each group of GRP chunks is ONE raw-byte DMA (bf16 slices read through
   bitcast views); all bulk groups go on the single gpsimd SWDGE queue so
   they complete strictly in order at the ~435 GB/s SBUF-fabric ceiling
   (spread across queues, the engines' packet round-robin would finish
   concurrent groups simultaneously, stalling compute). Small leading
   groups let compute start early; deep buffering (KV_BUFS) keeps the
   rings full so the tail doesn't expose per-descriptor latency.
 - Per chunk on device: scores[l,hq] = K_h^T.T @ qT (8 matmuls) into the
   group's PSUM batch tile; one exp per group on ScalarE; PV accumulation
   acc[hq,d+1] += E_h.T @ V_h (8 col-tiled matmuls into two PSUM banks),
   with the PV of group g emitted after QK of group g+1 so the PE never
   stalls on ScalarE. Slot drains go through VectorE into ring-buffered
   staging windows flushed mid-stream (a single end-of-kernel output DMA
   would crawl on drained rings, and per-slot output DMAs would share
   completion-semaphore lanes with the bulk loads and stall them).
"""

import os
import random
import sys
from contextlib import ExitStack

import numpy as np
import ml_dtypes  # noqa: F401  (numpy bf16/fp8 dtypes)

for _p in ("/opt/trn_rl_repo", "/root/.axon_site/_ro/trn_rl_repo"):
    if os.path.isdir(_p) and _p not in sys.path:
        sys.path.insert(0, _p)
        break

import concourse.bass as bass  # noqa: F401
import concourse.tile as tile
from concourse import bacc, mybir
from concourse.bass_utils import run_bass_kernel_spmd

B, HQ, HKV, D, BS, MB = 64, 32, 8, 128, 16, 128
G = HQ // HKV              # 4 query heads per kv head
SCALE = 0.08838834764831845
NCORES = 8
CHUNK = 128                # tokens per chunk (= SBUF partitions)
BPC = CHUNK // BS          # blocks per chunk = 8
ROW = HKV * D              # 1024 K elements per token row
DV = D + 1                 # V head row + denominator indicator column
ROWV = HKV * DV            # 1032 V elements per token row
GRP = 8                    # max chunks per combined K+V DMA group
KV_BUFS = 8                # raw-byte group tiles in flight
K_DT = "bf16"              # K/q dtype
V_DT = "bf16"              # V/E dtype
F8_REQS = 40               # the longest F8_REQS requests carry K and V in
                           # fp8e4m3 (mixed-dtype matmuls vs bf16 q/E).
                           # Long requests average quantization noise over
                           # many tokens: sim rel err 1.48e-2 vs the 2e-2
                           # gate, for -42.6% DMA bytes.

last_results = None        # stashed BassKernelResults for test.py

_prog_cache = {}
_sched_cache = {}


def _mdt(name):
    return {"f32": mybir.dt.float32, "bf16": mybir.dt.bfloat16,
            "fp8": mybir.dt.float8e4}[name]


def _ndt(name):
    return mybir.dt.np(_mdt(name))


def _group_sizes(C_total):
    """Small leading groups (fast compute start), 8-chunk steady state,
    small trailing groups (the last group's descriptors drain with shallow
    rings and exposed per-descriptor latency — keep it tiny)."""
    lead, tail = [], []
    for s in (2, 2, 4):
        if sum(lead) + s <= C_total - GRP:
            lead.append(s)
    for s in (2, 1):
        if sum(lead) + sum(tail) + s <= C_total - GRP:
            tail.insert(0, s)
    rest = C_total - sum(lead) - sum(tail)
    sizes = lead + [GRP] * (rest // GRP)
    if rest % GRP:
        sizes.append(rest % GRP)
    return sizes + tail


def _pack(pattern, sizes):
    """Assign request pieces to the 8*len(pattern) bins (desc order).

    Returns assignment dict (core, slot) -> (req, chunk_lo, n) or None if the
    pattern cannot hold all chunks."""
    order = sorted(range(len(pattern)), key=lambda r: -pattern[r])
    rem = sorted(((int(sizes[b]), b) for b in range(len(sizes))),
                 reverse=True)
    assign = {}
    for r in order:
        s = pattern[r]
        for c in range(NCORES):
            if not rem:
                assign[(c, r)] = None
                continue
            if rem[0][0] >= s:
                n, b = rem.pop(0)
                lo = int(sizes[b]) - n
                assign[(c, r)] = (b, lo, s)
                if n > s:
                    import bisect
                    bisect.insort_right(rem, (n - s, b))
                    rem.sort(reverse=True)
            else:
                hit = next((i for i, (n, _) in enumerate(rem) if n == s), 0)
                n, b = rem.pop(hit)
                lo = int(sizes[b]) - n
                assign[(c, r)] = (b, lo, n)
    if rem:
        return None
    return assign


def _find_pattern(sizes):
    """Local search for a slot-size pattern minimizing total chunks."""
    szs = sorted((int(s) for s in sizes), reverse=True)
    pat0 = tuple(szs[NCORES * r] for r in range(len(szs) // NCORES))
    best = (sum(pat0), pat0)
    rng = random.Random(1234)
    cur = list(pat0)
    lower = (sum(szs) + NCORES - 1) // NCORES
    for _ in range(15000):
        pat = cur[:]
        mv = rng.random()
        if mv < 0.4 and len(pat) > 2:
            i = rng.randrange(len(pat))
            if pat[i] > 1:
                pat[i] -= 1
            else:
                pat.pop(i)
        elif mv < 0.7:
            i = rng.randrange(len(pat))
            pat[i] += 1
        elif mv < 0.85 and len(pat) < 16:
            pat.append(rng.randint(1, 6))
        else:
            i = rng.randrange(len(pat))
            if pat[i] > 1:
                k = rng.randint(1, pat[i] - 1)
                pat[i] -= k
                pat.append(k)
        pat = tuple(sorted((p for p in pat if p > 0), reverse=True))
        if not pat or len(pat) > 16:
            continue
        if _pack(pat, sizes) is not None:
            if sum(pat) < best[0]:
                best = (sum(pat), pat)
                cur = list(pat)
                if best[0] <= lower:
                    break
            elif rng.random() < 0.3:
                cur = list(pat)
    return best[1]


def _make_schedule(context_lens):
    """Split requests into an fp8 set (longest F8_REQS) and a bf16 set; pack
    each into its own slots. Returns (pattern, nA, assign) where slots
    [0, nA) hold fp8 pieces."""
    key = context_lens.tobytes()
    if key not in _sched_cache:
        sizes = np.maximum(1, -(-context_lens.astype(np.int64) // CHUNK))
        order = np.argsort(-sizes, kind="stable")
        setA = order[:F8_REQS]
        setB = order[F8_REQS:]

        def sub_schedule(ids):
            if len(ids) == 0:
                return (), {}
            pat = _find_pattern(sizes[ids])
            amap = _pack(pat, sizes[ids])
            out = {}
            for (c, r), piece in amap.items():
                if piece is not None:
                    piece = (int(ids[piece[0]]), piece[1], piece[2])
                out[(c, r)] = piece
            return pat, out

        patA, asgA = sub_schedule(setA)
        patB, asgB = sub_schedule(setB)
        # interleave fp8 and bf16 slots proportionally so every DMA group
        # carries a similar bf16/fp8 mix (keeps both SBUF pools small)
        chA, chB = sum(patA), sum(patB)
        merged = []   # (src, local_idx)
        ia = ib = 0
        ca = cb = 0
        while ia < len(patA) or ib < len(patB):
            takeA = ib >= len(patB) or (
                ia < len(patA) and ca * chB <= cb * chA)
            if takeA:
                merged.append(("A", ia))
                ca += patA[ia]
                ia += 1
            else:
                merged.append(("B", ib))
                cb += patB[ib]
                ib += 1
        pattern = tuple(patA[i] if s == "A" else patB[i] for s, i in merged)
        f8mask = tuple(s == "A" for s, _ in merged)
        slotmap = {}
        for newr, (src, i) in enumerate(merged):
            slotmap[(src, i)] = newr
        assign = {}
        for (c, r), piece in asgA.items():
            assign[(c, slotmap[("A", r)])] = piece
        for (c, r), piece in asgB.items():
            assign[(c, slotmap[("B", r)])] = piece
        _sched_cache[key] = (pattern, f8mask, assign)
    return _sched_cache[key]


def _chunk_f8(pattern, f8mask):
    """Per-chunk fp8 classification from the per-slot fp8 mask."""
    isf8 = []
    for r, s in enumerate(pattern):
        isf8 += [bool(f8mask[r])] * s
    return isf8


def _build_program(pattern, f8mask):
    f32 = mybir.dt.float32
    kdt, vdt = _mdt(K_DT), _mdt(V_DT)
    SLOTS = len(pattern)
    C_total = sum(pattern)
    isf8 = _chunk_f8(pattern, f8mask)
    gsizes = _group_sizes(C_total)
    gidx0 = np.cumsum([0] + gsizes)
    # per-group byte layout in ONE raw blob (fp8-typed, bf16 read through
    # bitcast views): [bf K chunks (2B) | bf V chunks (2B) | f8 K | f8 V]
    gby = [0]
    for g, g_sz in enumerate(gsizes):
        nf8 = sum(1 for bi in range(g_sz) if isf8[int(gidx0[g]) + bi])
        nbf = g_sz - nf8
        gby.append(gby[-1] + 2 * nbf * (ROW + ROWV) + nf8 * (ROW + ROWV))
    slot_of = []
    for r, s in enumerate(pattern):
        slot_of += [r] * s
    nc = bacc.Bacc()

    kv_d = nc.declare_dram_parameter("kv", [D, int(gby[-1])],
                                     mybir.dt.float8e4, isOutput=False)
    qT_d = nc.declare_dram_parameter("qT", [D, SLOTS * HQ], kdt, isOutput=False)
    out_d = nc.declare_dram_parameter("out", [CHUNK, SLOTS * 2 * DV], f32,
                                      isOutput=True)

    EXP = mybir.ActivationFunctionType.Exp

    with tile.TileContext(nc) as tc, ExitStack() as ctx:
        kvpool = ctx.enter_context(tc.tile_pool(name="kvp", bufs=KV_BUFS))
        epool = ctx.enter_context(tc.tile_pool(name="e", bufs=3))
        const = ctx.enter_context(tc.tile_pool(name="cst", bufs=1))
        spsum = ctx.enter_context(tc.tile_pool(name="sp", bufs=2, space="PSUM"))
        apsum = ctx.enter_context(tc.tile_pool(name="ac", bufs=3, space="PSUM"))

        q_all = const.tile([D, SLOTS * HQ], kdt)
        nc.sync.dma_start(q_all[:], qT_d[:])
        ostpool = ctx.enter_context(tc.tile_pool(name="ost", bufs=2))
        OW = max(1, (SLOTS + 3) // 4)  # slots per output flush window
        # dummy matmul absorbs the q_all DMA wait so the first real matmul
        # only waits on its k/v DMA.
        dmy = spsum.tile([1, 1], f32, tag="sco")
        nc.tensor.matmul(dmy[:], q_all[0:1, 0:1], q_all[0:1, 0:1],
                         start=True, stop=True)

        accs = {}
        ost = {"tile": None, "start": 0}

        def get_acc(r):
            if r not in accs:
                accs[r] = (apsum.tile([CHUNK, DV], f32, tag="acca",
                                      name="acca"),
                           apsum.tile([CHUNK, DV], f32, tag="accb",
                                      name="accb"))
            return accs[r]

        def drain_slot(r):
            # slots drain in order into ring-buffered staging windows that
            # flush mid-stream, keeping the final output DMA small
            acc_a, acc_b = accs.pop(r)
            if ost["tile"] is None:
                ost["tile"] = ostpool.tile([CHUNK, OW * 2 * DV], f32,
                                           tag="ost", name="ost")
                ost["start"] = r
            o = (r - ost["start"]) * 2 * DV
            st = ost["tile"]
            nc.vector.tensor_copy(st[:, o:o + DV], acc_a[:])
            nc.vector.tensor_copy(st[:, o + DV:o + 2 * DV], acc_b[:])
            if r == SLOTS - 1 or r - ost["start"] == OW - 1:
                lo = ost["start"] * 2 * DV
                n = (r + 1 - ost["start"]) * 2 * DV
                nc.sync.dma_start(out_d[:, lo:lo + n], st[:, :n])
                ost["tile"] = None

        def emit_pv(pend):
            idx0, g_sz, et, vsl = pend
            for bi in range(g_sz):
                idx = idx0 + bi
                vt = vsl[bi]
                r = slot_of[idx]
                st = idx == 0 or slot_of[idx - 1] != r
                sp = idx == C_total - 1 or slot_of[idx + 1] != r
                acc_a, acc_b = get_acc(r)
                for h in range(HKV):
                    accp = acc_a if h < 4 else acc_b
                    jj = h % 4
                    nc.tensor.matmul(
                        accp[32 * jj:32 * jj + G, :],
                        et[:, bi * HQ + h * G:bi * HQ + (h + 1) * G],
                        vt[:, h * DV:(h + 1) * DV],
                        start=st, stop=sp,
                        tile_position=(0, 32 * jj),
                    )
                if sp:
                    drain_slot(r)

        gstart = [int(x) for x in gidx0[:-1]]

        def issue_group(g):
            """Allocate tiles for group g and issue its DMAs (gpsimd queue:
            per-queue FIFO completes groups in order; spreading across
            queues would finish concurrent groups simultaneously)."""
            g_sz = gsizes[g]
            i0 = gstart[g]
            f8s = [bi for bi in range(g_sz) if isf8[i0 + bi]]
            bfs = [bi for bi in range(g_sz) if not isf8[i0 + bi]]
            nbf, nf8 = len(bfs), len(f8s)
            nby = 2 * nbf * (ROW + ROWV) + nf8 * (ROW + ROWV)
            kv_t = kvpool.tile([D, nby], mybir.dt.float8e4,
                               tag="kvg", name="kvg")
            nc.gpsimd.dma_start(
                kv_t[:], kv_d[:, int(gby[g]):int(gby[g + 1])])
            ksl, vsl = {}, {}
            for p, bi in enumerate(bfs):
                ksl[bi] = kv_t[:, 2 * p * ROW:2 * (p + 1) * ROW] \
                    .bitcast(kdt)
                vo = 2 * nbf * ROW
                vsl[bi] = kv_t[:, vo + 2 * p * ROWV:vo + 2 * (p + 1) * ROWV] \
                    .bitcast(vdt)
            f8o = 2 * nbf * (ROW + ROWV)
            for p, bi in enumerate(f8s):
                ksl[bi] = kv_t[:, f8o + p * ROW:f8o + (p + 1) * ROW]
                vo = f8o + nf8 * ROW
                vsl[bi] = kv_t[:, vo + p * ROWV:vo + (p + 1) * ROWV]
            return ksl, vsl

        issued = {}

        # software-pipelined: QK+exp of group g+1 are emitted before PV of
        # group g, so the PE never stalls waiting for ScalarE's exp.
        pending = None
        idx0 = 0
        for g, g_sz in enumerate(gsizes):
            if g not in issued:
                issued[g] = issue_group(g)
            ksl, vsl = issued[g]
            sco = spsum.tile([CHUNK, g_sz * HQ], f32, tag="sco")
            for bi in range(g_sz):
                r = slot_of[idx0 + bi]
                kt = ksl[bi]
                qt = q_all[:, r * HQ:(r + 1) * HQ]  # noqa: F841 (clarity)
                for h in range(HKV):
                    nc.tensor.matmul(
                        sco[:, bi * HQ + h * G:bi * HQ + (h + 1) * G],
                        kt[:, h * D:(h + 1) * D],
                        qt[:, h * G:(h + 1) * G],
                        start=True, stop=True,
                    )
            et = epool.tile([CHUNK, g_sz * HQ], vdt)
            nc.scalar.activation(et[:], sco[:], EXP, bias=0.0, scale=1.0)
            if pending is not None:
                emit_pv(pending)
            pending = (idx0, g_sz, et, vsl)
            idx0 += g_sz
        emit_pv(pending)
    nc.compile()
    return nc


def _get_program(pattern, f8mask):
    if (pattern, f8mask) not in _prog_cache:
        _prog_cache[(pattern, f8mask)] = _build_program(pattern, f8mask)
    return _prog_cache[(pattern, f8mask)]


def _build_in_maps(q, k_cache, v_cache, block_tables, L, pattern, f8mask,
                   assign):
    np_k, np_v = _ndt(K_DT), _ndt(V_DT)
    np_f8 = _ndt("fp8")
    SLOTS = len(pattern)
    C_total = sum(pattern)
    isf8 = _chunk_f8(pattern, f8mask)
    gsizes = _group_sizes(C_total)
    nblocks_total = k_cache.shape[0]
    kf = k_cache.reshape(nblocks_total, BS, ROW)
    vf = v_cache.reshape(nblocks_total, BS, HKV, D)

    in_maps = []
    for c in range(NCORES):
        karr = np.empty((C_total, D, ROW), np_k)
        varr = np.zeros((C_total, CHUNK, HKV, DV), np_v)
        qT = np.zeros((D, SLOTS * HQ), np_k)
        gc = 0
        for r, s in enumerate(pattern):
            piece = assign[(c, r)]
            if piece is None:
                # fully padded slot: K from block 0, V stays zero
                blocks = np.zeros(s * BPC, np.int64)
                kreq = kf[blocks].reshape(s, CHUNK, HKV, D)
                karr[gc:gc + s] = \
                    kreq.transpose(0, 3, 2, 1).reshape(s, D, ROW)
                gc += s
                continue
            b, lo, n = piece
            bidx = np.clip(np.arange(lo * BPC, (lo + s) * BPC), 0, MB - 1)
            blocks = np.clip(block_tables[b, bidx].astype(np.int64),
                             0, nblocks_total - 1)
            kreq = kf[blocks].reshape(s, CHUNK, HKV, D)
            karr[gc:gc + s] = kreq.transpose(0, 3, 2, 1).reshape(s, D, ROW)
            # valid tokens of this piece: global idx in [lo*CHUNK, L_b)
            nval = min(n * CHUNK, max(0, int(L[b]) - lo * CHUNK))
            if nval > 0:
                vreq = vf[blocks[:n * BPC]].reshape(n * CHUNK, HKV, D)
                va = varr[gc:gc + s].reshape(s * CHUNK, HKV, DV)
                va[:nval, :, :D] = vreq[:nval]
                va[:nval, :, D] = 1.0
            qT[:, r * HQ:(r + 1) * HQ] = (q[b] * SCALE).T
            gc += s
        varr2 = varr.reshape(C_total, CHUNK, ROWV)
        parts = []   # uint8 blocks per group
        gc2 = 0
        for g_sz in gsizes:
            f8s = [bi for bi in range(g_sz) if isf8[gc2 + bi]]
            bfs = [bi for bi in range(g_sz) if not isf8[gc2 + bi]]
            for sel, dt in ((bfs, np_k), (f8s, np_f8)):
                if not sel:
                    continue
                ksel = karr[[gc2 + bi for bi in sel]].astype(dt)
                vsel = varr2[[gc2 + bi for bi in sel]].astype(dt)
                parts.append(ksel.transpose(1, 0, 2)
                             .reshape(D, len(sel) * ROW).view(np.uint8))
                parts.append(vsel.transpose(1, 0, 2)
                             .reshape(D, len(sel) * ROWV).view(np.uint8))
            gc2 += g_sz
        kvh = np.ascontiguousarray(np.concatenate(parts, axis=1)) \
            .view(np_f8)
        in_maps.append({"kv": kvh, "qT": qT})
    return in_maps


def kernel(q, k_cache, v_cache, block_tables, context_lens):
    global last_results
    q = np.asarray(q, dtype=np.float32)
    k_cache = np.asarray(k_cache, dtype=np.float32)
    v_cache = np.asarray(v_cache, dtype=np.float32)
    block_tables = np.asarray(block_tables, dtype=np.int32)
    context_lens = np.asarray(context_lens, dtype=np.int32)

    L = context_lens.astype(np.int64)
    pattern, f8mask, assign = _make_schedule(context_lens)
    SLOTS = len(pattern)
    nc = _get_program(pattern, f8mask)
    in_maps = _build_in_maps(
        q, k_cache, v_cache, block_tables, L, pattern, f8mask, assign)

    res = run_bass_kernel_spmd(
        nc, in_maps, list(range(NCORES)),
        trace=bool(os.environ.get("KBASS_TRACE")),
    )
    last_results = res

    num = np.zeros((B, HQ, D), np.float64)
    den = np.zeros((B, HQ, 1), np.float64)
    for c in range(NCORES):
        full = res.results[c]["out"].reshape(CHUNK, SLOTS, 2, DV) \
            .transpose(1, 2, 0, 3)
        for r in range(SLOTS):
            piece = assign[(c, r)]
            if piece is None:
                continue
            b = piece[0]
            for h in range(HKV):
                jj = h % 4
                strip = full[r, 0 if h < 4 else 1, 32 * jj:32 * jj + G, :]
                num[b, h * G:(h + 1) * G] += strip[:, :D]
                den[b, h * G:(h + 1) * G, 0] += strip[:, D]
    out = (num / np.maximum(den, 1e-30)).astype(np.float32)
    return out


# revision 47
# speedup vs baseline: 1.0118x; 1.0118x over previous
"""Paged GQA decode attention (B=64, HQ=32, HKV=8, D=128) on 8 TRN2 NeuronCores.

Strategy: data-parallel over request PIECES with host-side bin packing and
per-request mixed precision.
 - Each core runs the same static program: a sequence of slots with sizes
   `pattern` (in 128-token chunks). A slot accumulates attention for ONE
   contiguous piece of one request; long requests split into several pieces
   (across slots and cores) and the host sums the partial acc/den afterwards,
   which softmax's linearity over a shared exp-shift makes exact.
 - Bin packing (largest-bins-first, split-largest / exact-fit greedy over a
   locally-searched slot-size pattern) brings the per-core chunk count to the
   ceil(total/8) optimum, minimizing DMA bytes.
 - The longest F8_REQS requests carry K and V in fp8e4m3 (their many-token
   softmax averages quantization noise away; q and E stay bf16 via
   mixed-dtype matmuls), packed into dedicated fp8 slots interleaved with
   the bf16 slots so every DMA group has a similar byte mix.
 - Host gathers each piece's KV blocks (honoring block_tables) into per-core
   shards: K pre-transposed to [d, l] tiles, V natural [l, d]. The token
   validity mask is folded into V host-side (invalid rows zeroed) and each
   kv head's V carries a 129th column holding the validity indicator, so the
   PV matmul also accumulates the softmax denominator and exp needs no
   bias/mask at all.
 - Each group of GRP chunks is ONE raw-byte DMA (bf16 slices read through
   bitcast views); all bulk groups go on the single gpsimd SWDGE queue so
   they complete strictly in order at the ~435 GB/s SBUF-fabric ceiling
   (spread across queues, the engines' packet round-robin would finish
   concurrent groups simultaneously, stalling compute). Small leading
   groups let compute start early; deep buffering (KV_BUFS) keeps the
   rings full so the tail doesn't expose per-descriptor latency.
 - Per chunk on device: scores[l,hq] = K_h^T.T @ qT (8 matmuls) into the
   group's PSUM batch tile; one exp per group on ScalarE; PV accumulation
   acc[hq,d+1] += E_h.T @ V_h (8 col-tiled matmuls into two PSUM banks),
   with the PV of group g emitted after QK of group g+1 so the PE never
   stalls on ScalarE. Slot drains go through VectorE into ring-buffered
   staging windows flushed mid-stream (a single end-of-kernel output DMA
   would crawl on drained rings, and per-slot output DMAs would share
   completion-semaphore lanes with the bulk loads and stall them).
"""

import os
import random
import sys
from contextlib import ExitStack

import numpy as np
import ml_dtypes  # noqa: F401  (numpy bf16/fp8 dtypes)

for _p in ("/opt/trn_rl_repo", "/root/.axon_site/_ro/trn_rl_repo"):
    if os.path.isdir(_p) and _p not in sys.path:
        sys.path.insert(0, _p)
        break

import concourse.bass as bass  # noqa: F401
import concourse.tile as tile
from concourse import bacc, mybir
from concourse.bass_utils import run_bass_kernel_spmd

B, HQ, HKV, D, BS, MB = 64, 32, 8, 128, 16, 128
G = HQ // HKV              # 4 query heads per kv head
SCALE = 0.08838834764831845
NCORES = 8
CHUNK = 128                # tokens per chunk (= SBUF partitions)
BPC = CHUNK // BS          # blocks per chunk = 8
ROW = HKV * D              # 1024 K elements per token row
DV = D + 1                 # V head row + denominator indicator column
ROWV = HKV * DV            # 1032 V elements per token row
GRP = 8                    # max chunks per combined K+V DMA group
KV_BUFS = 8                # raw-byte group tiles in flight
K_DT = "bf16"              # K/q dtype
V_DT = "bf16"              # V/E dtype
F8_REQS = 40               # the longest F8_REQS requests carry K and V in
                           # fp8e4m3 (mixed-dtype matmuls vs bf16 q/E).
                           # Long requests average quantization noise over
                           # many tokens: sim rel err 1.48e-2 vs the 2e-2
                           # gate, for -42.6% DMA bytes.

last_results = None        # stashed BassKernelResults for test.py

_prog_cache = {}
_sched_cache = {}


def _mdt(name):
    return {"f32": mybir.dt.float32, "bf16": mybir.dt.bfloat16,
            "fp8": mybir.dt.float8e4}[name]


def _ndt(name):
    return mybir.dt.np(_mdt(name))


def _group_sizes(C_total):
    """Small leading groups (fast compute start), 8-chunk steady state,
    small trailing groups (the last group's descriptors drain with shallow
    rings and exposed per-descriptor latency — keep it tiny)."""
    lead, tail = [], []
    for s in (2, 2, 4):
        if sum(lead) + s <= C_total - GRP:
            lead.append(s)
    for s in (2, 1):
        if sum(lead) + sum(tail) + s <= C_total - GRP:
            tail.insert(0, s)
    rest = C_total - sum(lead) - sum(tail)
    sizes = lead + [GRP] * (rest // GRP)
    if rest % GRP:
        sizes.append(rest % GRP)
    return sizes + tail


def _pack(pattern, sizes):
    """Assign request pieces to the 8*len(pattern) bins (desc order).

    Returns assignment dict (core, slot) -> (req, chunk_lo, n) or None if the
    pattern cannot hold all chunks."""
    order = sorted(range(len(pattern)), key=lambda r: -pattern[r])
    rem = sorted(((int(sizes[b]), b) for b in range(len(sizes))),
                 reverse=True)
    assign = {}
    for r in order:
        s = pattern[r]
        for c in range(NCORES):
            if not rem:
                assign[(c, r)] = None
                continue
            if rem[0][0] >= s:
                n, b = rem.pop(0)
                lo = int(sizes[b]) - n
                assign[(c, r)] = (b, lo, s)
                if n > s:
                    import bisect
                    bisect.insort_right(rem, (n - s, b))
                    rem.sort(reverse=True)
            else:
                hit = next((i for i, (n, _) in enumerate(rem) if n == s), 0)
                n, b = rem.pop(hit)
                lo = int(sizes[b]) - n
                assign[(c, r)] = (b, lo, n)
    if rem:
        return None
    return assign


def _find_pattern(sizes):
    """Local search for a slot-size pattern minimizing total chunks."""
    szs = sorted((int(s) for s in sizes), reverse=True)
    pat0 = tuple(szs[NCORES * r] for r in range(len(szs) // NCORES))
    best = (sum(pat0), pat0)
    rng = random.Random(1234)
    cur = list(pat0)
    lower = (sum(szs) + NCORES - 1) // NCORES
    for _ in range(15000):
        pat = cur[:]
        mv = rng.random()
        if mv < 0.4 and len(pat) > 2:
            i = rng.randrange(len(pat))
            if pat[i] > 1:
                pat[i] -= 1
            else:
                pat.pop(i)
        elif mv < 0.7:
            i = rng.randrange(len(pat))
            pat[i] += 1
        elif mv < 0.85 and len(pat) < 16:
            pat.append(rng.randint(1, 6))
        else:
            i = rng.randrange(len(pat))
            if pat[i] > 1:
                k = rng.randint(1, pat[i] - 1)
                pat[i] -= k
                pat.append(k)
        pat = tuple(sorted((p for p in pat if p > 0), reverse=True))
        if not pat or len(pat) > 16:
            continue
        if _pack(pat, sizes) is not None:
            if sum(pat) < best[0]:
                best = (sum(pat), pat)
                cur = list(pat)
                if best[0] <= lower:
                    break
            elif rng.random() < 0.3:
                cur = list(pat)
    return best[1]


def _make_schedule(context_lens):
    """Split requests into an fp8 set (longest F8_REQS) and a bf16 set; pack
    each into its own slots. Returns (pattern, nA, assign) where slots
    [0, nA) hold fp8 pieces."""
    key = context_lens.tobytes()
    if key not in _sched_cache:
        sizes = np.maximum(1, -(-context_lens.astype(np.int64) // CHUNK))
        order = np.argsort(-sizes, kind="stable")
        setA = order[:F8_REQS]
        setB = order[F8_REQS:]

        def sub_schedule(ids):
            if len(ids) == 0:
                return (), {}
            pat = _find_pattern(sizes[ids])
            amap = _pack(pat, sizes[ids])
            out = {}
            for (c, r), piece in amap.items():
                if piece is not None:
                    piece = (int(ids[piece[0]]), piece[1], piece[2])
                out[(c, r)] = piece
            return pat, out

        patA, asgA = sub_schedule(setA)
        patB, asgB = sub_schedule(setB)
        # interleave fp8 and bf16 slots proportionally so every DMA group
        # carries a similar bf16/fp8 mix (keeps both SBUF pools small)
        chA, chB = sum(patA), sum(patB)
        merged = []   # (src, local_idx)
        ia = ib = 0
        ca = cb = 0
        while ia < len(patA) or ib < len(patB):
            takeA = ib >= len(patB) or (
                ia < len(patA) and ca * chB <= cb * chA)
            if takeA:
                merged.append(("A", ia))
                ca += patA[ia]
                ia += 1
            else:
                merged.append(("B", ib))
                cb += patB[ib]
                ib += 1
        pattern = tuple(patA[i] if s == "A" else patB[i] for s, i in merged)
        f8mask = tuple(s == "A" for s, _ in merged)
        slotmap = {}
        for newr, (src, i) in enumerate(merged):
            slotmap[(src, i)] = newr
        assign = {}
        for (c, r), piece in asgA.items():
            assign[(c, slotmap[("A", r)])] = piece
        for (c, r), piece in asgB.items():
            assign[(c, slotmap[("B", r)])] = piece
        _sched_cache[key] = (pattern, f8mask, assign)
    return _sched_cache[key]


def _chunk_f8(pattern, f8mask):
    """Per-chunk fp8 classification from the per-slot fp8 mask."""
    isf8 = []
    for r, s in enumerate(pattern):
        isf8 += [bool(f8mask[r])] * s
    return isf8


def _build_program(pattern, f8mask):
    f32 = mybir.dt.float32
    kdt, vdt = _mdt(K_DT), _mdt(V_DT)
    SLOTS = len(pattern)
    C_total = sum(pattern)
    isf8 = _chunk_f8(pattern, f8mask)
    gsizes = _group_sizes(C_total)
    gidx0 = np.cumsum([0] + gsizes)
    # per-group byte layout in ONE raw blob (fp8-typed, bf16 read through
    # bitcast views): [bf K chunks (2B) | bf V chunks (2B) | f8 K | f8 V]
    gby = [0]
    for g, g_sz in enumerate(gsizes):
        nf8 = sum(1 for bi in range(g_sz) if isf8[int(gidx0[g]) + bi])
        nbf = g_sz - nf8
        gby.append(gby[-1] + 2 * nbf * (ROW + ROWV) + nf8 * (ROW + ROWV))
    slot_of = []
    for r, s in enumerate(pattern):
        slot_of += [r] * s
    nc = bacc.Bacc()

    kv_d = nc.declare_dram_parameter("kv", [D, int(gby[-1])],
                                     mybir.dt.float8e4, isOutput=False)
    qT_d = nc.declare_dram_parameter("qT", [D, SLOTS * HQ], kdt, isOutput=False)
    out_d = nc.declare_dram_parameter("out", [CHUNK, SLOTS * 2 * DV], f32,
                                      isOutput=True)

    EXP = mybir.ActivationFunctionType.Exp

    with tile.TileContext(nc) as tc, ExitStack() as ctx:
        kvpool = ctx.enter_context(tc.tile_pool(name="kvp", bufs=KV_BUFS))
        epool = ctx.enter_context(tc.tile_pool(name="e", bufs=3))
        const = ctx.enter_context(tc.tile_pool(name="cst", bufs=1))
        spsum = ctx.enter_context(tc.tile_pool(name="sp", bufs=2, space="PSUM"))
        apsum = ctx.enter_context(tc.tile_pool(name="ac", bufs=3, space="PSUM"))

        q_all = const.tile([D, SLOTS * HQ], kdt)
        nc.sync.dma_start(q_all[:], qT_d[:])
        ostpool = ctx.enter_context(tc.tile_pool(name="ost", bufs=2))
        OW = max(1, (SLOTS + 3) // 4)  # slots per output flush window
        # dummy matmul absorbs the q_all DMA wait so the first real matmul
        # only waits on its k/v DMA.
        dmy = spsum.tile([1, 1], f32, tag="sco")
        nc.tensor.matmul(dmy[:], q_all[0:1, 0:1], q_all[0:1, 0:1],
                         start=True, stop=True)

        accs = {}
        ost = {"tile": None, "start": 0}

        def get_acc(r):
            if r not in accs:
                accs[r] = (apsum.tile([CHUNK, DV], f32, tag="acca",
                                      name="acca"),
                           apsum.tile([CHUNK, DV], f32, tag="accb",
                                      name="accb"))
            return accs[r]

        def drain_slot(r):
            # slots drain in order into ring-buffered staging windows that
            # flush mid-stream, keeping the final output DMA small
            acc_a, acc_b = accs.pop(r)
            if ost["tile"] is None:
                ost["tile"] = ostpool.tile([CHUNK, OW * 2 * DV], f32,
                                           tag="ost", name="ost")
                ost["start"] = r
            o = (r - ost["start"]) * 2 * DV
            st = ost["tile"]
            nc.vector.tensor_copy(st[:, o:o + DV], acc_a[:])
            nc.vector.tensor_copy(st[:, o + DV:o + 2 * DV], acc_b[:])
            if r == SLOTS - 1 or r - ost["start"] == OW - 1:
                lo = ost["start"] * 2 * DV
                n = (r + 1 - ost["start"]) * 2 * DV
                nc.sync.dma_start(out_d[:, lo:lo + n], st[:, :n])
                ost["tile"] = None

        def emit_pv(pend):
            idx0, g_sz, et, vsl = pend
            for bi in range(g_sz):
                idx = idx0 + bi
                vt = vsl[bi]
                r = slot_of[idx]
                st = idx == 0 or slot_of[idx - 1] != r
                sp = idx == C_total - 1 or slot_of[idx + 1] != r
                acc_a, acc_b = get_acc(r)
                for h in range(HKV):
                    accp = acc_a if h < 4 else acc_b
                    jj = h % 4
                    nc.tensor.matmul(
                        accp[32 * jj:32 * jj + G, :],
                        et[:, bi * HQ + h * G:bi * HQ + (h + 1) * G],
                        vt[:, h * DV:(h + 1) * DV],
                        start=st, stop=sp,
                        tile_position=(0, 32 * jj),
                    )
                if sp:
                    drain_slot(r)

        gstart = [int(x) for x in gidx0[:-1]]

        def issue_group(g):
            """Allocate tiles for group g and issue its DMAs (gpsimd queue:
            per-queue FIFO completes groups in order; spreading across
            queues would finish concurrent groups simultaneously)."""
            g_sz = gsizes[g]
            i0 = gstart[g]
            f8s = [bi for bi in range(g_sz) if isf8[i0 + bi]]
            bfs = [bi for bi in range(g_sz) if not isf8[i0 + bi]]
            nbf, nf8 = len(bfs), len(f8s)
            nby = 2 * nbf * (ROW + ROWV) + nf8 * (ROW + ROWV)
            kv_t = kvpool.tile([D, nby], mybir.dt.float8e4,
                               tag="kvg", name="kvg")
            nc.sync.dma_start(
                kv_t[:], kv_d[:, int(gby[g]):int(gby[g + 1])])
            ksl, vsl = {}, {}
            for p, bi in enumerate(bfs):
                ksl[bi] = kv_t[:, 2 * p * ROW:2 * (p + 1) * ROW] \
                    .bitcast(kdt)
                vo = 2 * nbf * ROW
                vsl[bi] = kv_t[:, vo + 2 * p * ROWV:vo + 2 * (p + 1) * ROWV] \
                    .bitcast(vdt)
            f8o = 2 * nbf * (ROW + ROWV)
            for p, bi in enumerate(f8s):
                ksl[bi] = kv_t[:, f8o + p * ROW:f8o + (p + 1) * ROW]
                vo = f8o + nf8 * ROW
                vsl[bi] = kv_t[:, vo + p * ROWV:vo + (p + 1) * ROWV]
            return ksl, vsl

        issued = {}

        # software-pipelined: QK+exp of group g+1 are emitted before PV of
        # group g, so the PE never stalls waiting for ScalarE's exp.
        pending = None
        idx0 = 0
        for g, g_sz in enumerate(gsizes):
            if g not in issued:
                issued[g] = issue_group(g)
            ksl, vsl = issued[g]
            sco = spsum.tile([CHUNK, g_sz * HQ], f32, tag="sco")
            for bi in range(g_sz):
                r = slot_of[idx0 + bi]
                kt = ksl[bi]
                qt = q_all[:, r * HQ:(r + 1) * HQ]  # noqa: F841 (clarity)
                for h in range(HKV):
                    nc.tensor.matmul(
                        sco[:, bi * HQ + h * G:bi * HQ + (h + 1) * G],
                        kt[:, h * D:(h + 1) * D],
                        qt[:, h * G:(h + 1) * G],
                        start=True, stop=True,
                    )
            et = epool.tile([CHUNK, g_sz * HQ], vdt)
            nc.scalar.activation(et[:], sco[:], EXP, bias=0.0, scale=1.0)
            if pending is not None:
                emit_pv(pending)
            pending = (idx0, g_sz, et, vsl)
            idx0 += g_sz
        emit_pv(pending)
    nc.compile()
    return nc


def _get_program(pattern, f8mask):
    if (pattern, f8mask) not in _prog_cache:
        _prog_cache[(pattern, f8mask)] = _build_program(pattern, f8mask)
    return _prog_cache[(pattern, f8mask)]


def _build_in_maps(q, k_cache, v_cache, block_tables, L, pattern, f8mask,
                   assign):
    np_k, np_v = _ndt(K_DT), _ndt(V_DT)
    np_f8 = _ndt("fp8")
    SLOTS = len(pattern)
    C_total = sum(pattern)
    isf8 = _chunk_f8(pattern, f8mask)
    gsizes = _group_sizes(C_total)
    nblocks_total = k_cache.shape[0]
    kf = k_cache.reshape(nblocks_total, BS, ROW)
    vf = v_cache.reshape(nblocks_total, BS, HKV, D)

    in_maps = []
    for c in range(NCORES):
        karr = np.empty((C_total, D, ROW), np_k)
        varr = np.zeros((C_total, CHUNK, HKV, DV), np_v)
        qT = np.zeros((D, SLOTS * HQ), np_k)
        gc = 0
        for r, s in enumerate(pattern):
            piece = assign[(c, r)]
            if piece is None:
                # fully padded slot: K from block 0, V stays zero
                blocks = np.zeros(s * BPC, np.int64)
                kreq = kf[blocks].reshape(s, CHUNK, HKV, D)
                karr[gc:gc + s] = \
                    kreq.transpose(0, 3, 2, 1).reshape(s, D, ROW)
                gc += s
                continue
            b, lo, n = piece
            bidx = np.clip(np.arange(lo * BPC, (lo + s) * BPC), 0, MB - 1)
            blocks = np.clip(block_tables[b, bidx].astype(np.int64),
                             0, nblocks_total - 1)
            kreq = kf[blocks].reshape(s, CHUNK, HKV, D)
            karr[gc:gc + s] = kreq.transpose(0, 3, 2, 1).reshape(s, D, ROW)
            # valid tokens of this piece: global idx in [lo*CHUNK, L_b)
            nval = min(n * CHUNK, max(0, int(L[b]) - lo * CHUNK))
            if nval > 0:
                vreq = vf[blocks[:n * BPC]].reshape(n * CHUNK, HKV, D)
                va = varr[gc:gc + s].reshape(s * CHUNK, HKV, DV)
                va[:nval, :, :D] = vreq[:nval]
                va[:nval, :, D] = 1.0
            qT[:, r * HQ:(r + 1) * HQ] = (q[b] * SCALE).T
            gc += s
        varr2 = varr.reshape(C_total, CHUNK, ROWV)
        parts = []   # uint8 blocks per group
        gc2 = 0
        for g_sz in gsizes:
            f8s = [bi for bi in range(g_sz) if isf8[gc2 + bi]]
            bfs = [bi for bi in range(g_sz) if not isf8[gc2 + bi]]
            for sel, dt in ((bfs, np_k), (f8s, np_f8)):
                if not sel:
                    continue
                ksel = karr[[gc2 + bi for bi in sel]].astype(dt)
                vsel = varr2[[gc2 + bi for bi in sel]].astype(dt)
                parts.append(ksel.transpose(1, 0, 2)
                             .reshape(D, len(sel) * ROW).view(np.uint8))
                parts.append(vsel.transpose(1, 0, 2)
                             .reshape(D, len(sel) * ROWV).view(np.uint8))
            gc2 += g_sz
        kvh = np.ascontiguousarray(np.concatenate(parts, axis=1)) \
            .view(np_f8)
        in_maps.append({"kv": kvh, "qT": qT})
    return in_maps


def kernel(q, k_cache, v_cache, block_tables, context_lens):
    global last_results
    q = np.asarray(q, dtype=np.float32)
    k_cache = np.asarray(k_cache, dtype=np.float32)
    v_cache = np.asarray(v_cache, dtype=np.float32)
    block_tables = np.asarray(block_tables, dtype=np.int32)
    context_lens = np.asarray(context_lens, dtype=np.int32)

    L = context_lens.astype(np.int64)
    pattern, f8mask, assign = _make_schedule(context_lens)
    SLOTS = len(pattern)
    nc = _get_program(pattern, f8mask)
    in_maps = _build_in_maps(
        q, k_cache, v_cache, block_tables, L, pattern, f8mask, assign)

    res = run_bass_kernel_spmd(
        nc, in_maps, list(range(NCORES)),
        trace=bool(os.environ.get("KBASS_TRACE")),
    )
    last_results = res

    num = np.zeros((B, HQ, D), np.float64)
    den = np.zeros((B, HQ, 1), np.float64)
    for c in range(NCORES):
        full = res.results[c]["out"].reshape(CHUNK, SLOTS, 2, DV) \
            .transpose(1, 2, 0, 3)
        for r in range(SLOTS):
            piece = assign[(c, r)]
            if piece is None:
                continue
            b = piece[0]
            for h in range(HKV):
                jj = h % 4
                strip = full[r, 0 if h < 4 else 1, 32 * jj:32 * jj + G, :]
                num[b, h * G:(h + 1) * G] += strip[:, :D]
                den[b, h * G:(h + 1) * G, 0] += strip[:, D]
    out = (num / np.maximum(den, 1e-30)).astype(np.float32)
    return out
